# revision 2
# baseline (speedup 1.0000x reference)
"""ChebConv (K=4) GNN layer on 8 Trainium2 NeuronCores — v2.

Design (replaces the ap_gather/PE-transpose pipeline, which was
GPSIMD-bound at ~22ns/edge):
  - Nodes V row-sharded across 8 cores (VS rows, padded to VSP).
  - The current poly y lives replicated in DRAM as row-major bf16
    [VG, D] (D = B*FIN = 256 -> 512B rows).
  - Per row-tile cell, gpsimd.dma_gather pulls the edges' source rows
    straight from DRAM into SBUF in edge-partitioned layout
    [128, ngroups, D] (SWDGE descriptors; no PE transpose needed).
  - Segment-sum via one-hot matmul: S[e, r] = val[e] * (rloc[e] == r)
    generated on-chip from iota + meta, contracted against the gathered
    rows, accumulating a [128, D] psum per row-tile across its cells.
  - Chebyshev recurrence per row-tile; new shard written row-major bf16
    and AllGathered to rebuild the replica for the next round.
  - Final einsum contracts T_k^T with the replicated weight on the PE.

SPMD: per-cell edge counts are padded to the max across cores, so the
instruction stream is identical on all cores; only idx/meta data differ.
"""

import sys

import numpy as np

sys.path.insert(0, "/opt/trn_rl_repo")

import ml_dtypes  # noqa: E402

BF16 = ml_dtypes.bfloat16


def make_cfg(V=100000, E=1600000, B=4, FIN=64, FOUT=64, NC=8, RT=128, NCH=4,
             NQ=1, PSB=4):
    VS = V // NC
    assert VS * NC == V
    VSP = ((VS + RT - 1) // RT) * RT
    NT = VSP // RT
    VG = VSP * NC
    assert VG % NCH == 0
    CHUNK = VG // NCH
    assert CHUNK <= 32767  # idx is int16
    D = B * FIN
    return dict(V=V, E=E, B=B, FIN=FIN, FOUT=FOUT, NC=NC, RT=RT, NCH=NCH,
                CHUNK=CHUNK, VS=VS, VSP=VSP, NT=NT, VG=VG, D=D, NQ=NQ,
                PSB=PSB)


def _wrap16(idx, npart=128):
    n = idx.shape[0]
    w = idx.reshape(n // 16, 16).T  # [16, n/16]
    return np.tile(w, (npart // 16, 1))


def preprocess(rows, cols, vals, cfg):
    """Static SPMD schedule + per-core idx/meta data.

    prog: per row-tile, list of (ch, n16, ioff, goff) cells.
    per_core[c]: gidx [128, NIDX/16] i16, meta [128, NG, 2] f32.
    """
    NC, VS, VSP, RT, NT = cfg["NC"], cfg["VS"], cfg["VSP"], cfg["RT"], cfg["NT"]
    CHUNK, NCH = cfg["CHUNK"], cfg["NCH"]

    rows = np.asarray(rows, dtype=np.int64)
    cols = np.asarray(cols, dtype=np.int64)
    vals = np.asarray(vals, dtype=np.float32)

    owner = rows // VS
    lr = rows - owner * VS
    rt = lr // RT
    rloc = lr - rt * RT
    gc = (cols // VS) * VSP + (cols % VS)  # padded-global source index
    ch = gc // CHUNK
    ci = (gc - ch * CHUNK).astype(np.int64)

    cell_of = rt * NCH + ch
    ncells = NT * NCH
    counts = np.zeros((NC, ncells), dtype=np.int64)
    for c in range(NC):
        counts[c] = np.bincount(cell_of[owner == c], minlength=ncells)
    mx = counts.max(axis=0)
    n16 = ((mx + 15) // 16) * 16
    n16v = n16.reshape(NT, NCH)
    for t in range(NT):
        if n16v[t].sum() == 0:
            n16v[t, 0] = 16  # keep the psum chain non-empty

    per_core_cells = []
    for c in range(NC):
        m = owner == c
        order = np.argsort(cell_of[m], kind="stable")
        e_ci = ci[m][order]
        e_rloc = rloc[m][order]
        e_val = vals[m][order]
        e_cell = cell_of[m][order]
        starts = np.searchsorted(e_cell, np.arange(ncells))
        ends = np.searchsorted(e_cell, np.arange(ncells) + 1)
        per_core_cells.append((e_ci, e_rloc, e_val, starts, ends))

    NIDX = int(n16v.sum())
    NG = int(((n16v + RT - 1) // RT).sum())

    gidx = [np.zeros(NIDX, dtype=np.int16) for _ in range(NC)]
    grloc = [np.zeros((NG, RT), dtype=np.float32) for _ in range(NC)]
    gval = [np.zeros((NG, RT), dtype=np.float32) for _ in range(NC)]

    prog = []
    ioff = 0
    goff = 0
    for t in range(NT):
        tcells = []
        for chv in range(NCH):
            n = int(n16v[t, chv])
            if n == 0:
                continue
            ngrp = (n + RT - 1) // RT
            for c in range(NC):
                e_ci, e_rloc, e_val, starts, ends = per_core_cells[c]
                s_, e_ = starts[t * NCH + chv], ends[t * NCH + chv]
                k = e_ - s_
                gidx[c][ioff:ioff + k] = e_ci[s_:e_].astype(np.int16)
                gr = grloc[c][goff:goff + ngrp].reshape(-1)
                gv = gval[c][goff:goff + ngrp].reshape(-1)
                gr[:k] = e_rloc[s_:e_].astype(np.float32)
                gv[:k] = e_val[s_:e_].astype(np.float32)
            tcells.append({"ch": chv, "n16": n, "ngrp": ngrp,
                           "ioff": ioff, "goff": goff})
            ioff += n
            goff += ngrp
        prog.append(tcells)
    assert ioff == NIDX and goff == NG

    per_core = []
    for c in range(NC):
        meta = np.zeros((128, NG, 2), dtype=np.float32)
        meta[:, :, 0] = grloc[c].T
        meta[:, :, 1] = gval[c].T
        per_core.append({"gidx": _wrap16(gidx[c]), "meta": meta})

    return {"NIDX": NIDX, "NG": NG, "tiles": prog}, per_core


def build_nc(cfg, prog, ag_mode="collective"):
    import concourse.bacc as bacc
    import concourse.mybir as mybir
    import concourse.tile as tile

    NC, VSP, VG, D, RT, NT = (cfg["NC"], cfg["VSP"], cfg["VG"], cfg["D"],
                              cfg["RT"], cfg["NT"])
    CHUNK, B, FIN, FOUT = cfg["CHUNK"], cfg["B"], cfg["FIN"], cfg["FOUT"]
    NCH, NQ, PSB = cfg["NCH"], cfg["NQ"], cfg["PSB"]
    NIDX, NG = prog["NIDX"], prog["NG"]
    f32, bf16, i16 = mybir.dt.float32, mybir.dt.bfloat16, mybir.dt.int16
    AG_GROUPS = [list(range(NC))]
    GMAX = max(c["ngrp"] for tc in prog["tiles"] for c in tc)

    nc = bacc.Bacc("TRN2", target_bir_lowering=False, debug=False,
                   num_devices=NC)

    # inputs
    xg0 = nc.dram_tensor("xg0", [VG, D], bf16, kind="ExternalInput")
    x0s = nc.dram_tensor("x0s", [VSP, D], f32, kind="ExternalInput")
    x0t = nc.dram_tensor("x0t", [D, VSP], f32, kind="ExternalInput")
    gidx_d = nc.dram_tensor("gidx", [128, NIDX // 16], i16, kind="ExternalInput")
    meta_d = nc.dram_tensor("meta", [128, NG, 2], f32, kind="ExternalInput")
    iota_d = nc.dram_tensor("iota", [128, RT], bf16, kind="ExternalInput")
    ident_d = nc.dram_tensor("ident", [128, 128], f32, kind="ExternalInput")
    w0_d = nc.dram_tensor("w0", [FIN, FOUT], f32, kind="ExternalInput")
    wb_d = nc.dram_tensor("wb", [FIN, 3, FOUT], bf16, kind="ExternalInput")
    bias_d = nc.dram_tensor("biasin", [FOUT, 1], f32, kind="ExternalInput")

    # outputs
    outT = nc.dram_tensor("outT", [B, FOUT, VSP], f32, kind="ExternalOutput")

    # internal DRAM
    xs1 = nc.dram_tensor("xs1", [VSP, D], f32)
    xb = [nc.dram_tensor(f"xb{k}", [VSP, D], bf16) for k in (1, 2)]
    yr = [nc.dram_tensor(f"yr{k}", [VG, D], bf16, addr_space="Shared")
          for k in (1, 2)]
    xt = [nc.dram_tensor(f"xt{k}", [D, VSP], bf16) for k in (1, 2, 3)]

    with tile.TileContext(nc) as tc:
        with (
            tc.tile_pool(name="static", bufs=1) as sp,
            tc.tile_pool(name="ztp", bufs=6) as ztp,
            tc.tile_pool(name="stp", bufs=8) as stp,
            tc.tile_pool(name="fin", bufs=3) as fp,
            tc.tile_pool(name="psum", bufs=1, space="PSUM") as pp,
            tc.tile_pool(name="pst", bufs=2, space="PSUM") as ppt,
        ):
            meta_t = sp.tile([128, NG, 2], f32)
            iota_t = sp.tile([128, RT], bf16)
            ident_t = sp.tile([128, 128], f32)
            gidx_t = sp.tile([128, NIDX // 16], i16)
            nc.sync.dma_start(out=meta_t[:], in_=meta_d[:])
            nc.sync.dma_start(out=iota_t[:], in_=iota_d[:])
            nc.sync.dma_start(out=ident_t[:], in_=ident_d[:])
            nc.sync.dma_start(out=gidx_t[:], in_=gidx_d[:])

            def spmm_round(r):
                src = xg0 if r == 1 else yr[r - 2]
                for t in range(NT):
                    cells = prog["tiles"][t]
                    tot = sum(c["ngrp"] for c in cells)
                    ps = pp.tile([128, D], f32, tag=f"ps{t % PSB}",
                                 name=f"ps_{r}_{t}")
                    g_i = 0
                    for cell in cells:
                        chv, n, ngrp = cell["ch"], cell["n16"], cell["ngrp"]
                        ioff, goff = cell["ioff"], cell["goff"]
                        zt = ztp.tile([128, GMAX, D], bf16, tag="zt",
                                      name=f"zt_{r}_{t}_{chv}")
                        nc.gpsimd.dma_gather(
                            zt[:, :ngrp, :],
                            src[chv * CHUNK:(chv + 1) * CHUNK, :],
                            gidx_t[:, ioff // 16:(ioff + n) // 16],
                            num_idxs=n, num_idxs_reg=n, elem_size=D,
                            queue_num=(t * NCH + chv) % NQ)
                        for g in range(ngrp):
                            # last group only has kk valid gathered rows;
                            # contract over those to avoid reading the rest
                            kk = min(128, n - g * 128)
                            st = stp.tile([128, RT], bf16, tag="st",
                                          name=f"st_{r}_{t}_{chv}_{g}")
                            nc.any.tensor_scalar(
                                st[:], iota_t[:],
                                meta_t[:, goff + g, 0:1],
                                meta_t[:, goff + g, 1:2],
                                op0=mybir.AluOpType.is_equal,
                                op1=mybir.AluOpType.mult)
                            g_i += 1
                            nc.tensor.matmul(
                                ps[:], st[:kk, :], zt[:kk, g, :],
                                start=(g_i == 1), stop=(g_i == tot))
                    # finalize row tile
                    xnew = fp.tile([128, D], f32, tag="xnew",
                                   name=f"xnew_{r}_{t}")
                    if r == 1:
                        nc.vector.tensor_copy(xnew[:], ps[:])
                        nc.sync.dma_start(
                            out=xs1[t * RT:(t + 1) * RT, :], in_=xnew[:])
                    else:
                        xprev_src = x0s if r == 2 else xs1
                        xp = fp.tile([128, D], f32, tag="xp",
                                     name=f"xp_{r}_{t}")
                        nc.sync.dma_start(
                            out=xp[:], in_=xprev_src[t * RT:(t + 1) * RT, :])
                        nc.vector.scalar_tensor_tensor(
                            xnew[:], ps[:], 2.0, xp[:],
                            op0=mybir.AluOpType.mult,
                            op1=mybir.AluOpType.subtract)
                    if r <= 2:
                        xnb = fp.tile([128, D], bf16, tag="xnb",
                                      name=f"xnb_{r}_{t}")
                        nc.scalar.copy(xnb[:], xnew[:])
                        nc.sync.dma_start(
                            out=xb[r - 1][t * RT:(t + 1) * RT, :], in_=xnb[:])
                    # transposed bf16 copy for the einsum
                    pa = ppt.tile([128, 128], f32, tag="pt",
                                  name=f"pa_{r}_{t}")
                    pb = ppt.tile([128, 128], f32, tag="pt",
                                  name=f"pb_{r}_{t}")
                    xe = xnew[:].rearrange("v (f two) -> v two f", two=2)
                    nc.tensor.transpose(pa[:], xe[:, 0, :], ident_t[:])
                    nc.tensor.transpose(pb[:], xe[:, 1, :], ident_t[:])
                    xtp = fp.tile([128, 2, 128], bf16, tag="xtp",
                                  name=f"xtp_{r}_{t}")
                    nc.any.tensor_copy(xtp[:, 0, :], pa[:])
                    nc.any.tensor_copy(xtp[:, 1, :], pb[:])
                    nc.sync.dma_start(
                        out=xt[r - 1].rearrange(
                            "(f two) v -> f two v",
                            two=2)[:, :, t * RT:(t + 1) * RT],
                        in_=xtp[:])
                if r <= 2:
                    if ag_mode == "collective":
                        nc.gpsimd.collective_compute(
                            "AllGather", mybir.AluOpType.bypass,
                            replica_groups=AG_GROUPS,
                            ins=[xb[r - 1][:]], outs=[yr[r - 1][:]])
                    else:  # single-core timing stand-in
                        for c in range(NC):
                            nc.sync.dma_start(
                                out=yr[r - 1][c * VSP:(c + 1) * VSP, :],
                                in_=xb[r - 1][:])

            for r in (1, 2, 3):
                spmm_round(r)

        # einsum: outT[b][o, v] = sum_k W_k^T @ T_k^T[b-rows, v] + bias
        with (
            tc.tile_pool(name="ew", bufs=1) as ewp,
            tc.tile_pool(name="erhs", bufs=3) as erp,
            tc.tile_pool(name="eout", bufs=3) as eop,
            tc.tile_pool(name="epsum", bufs=1, space="PSUM") as epp,
        ):
            w0_t = ewp.tile([FIN, FOUT], f32)
            wb_t = ewp.tile([FIN, 3, FOUT], bf16)
            bias_t = ewp.tile([FOUT, 1], f32)
            nc.sync.dma_start(out=w0_t[:], in_=w0_d[:])
            nc.sync.dma_start(out=wb_t[:], in_=wb_d[:])
            nc.sync.dma_start(out=bias_t[:], in_=bias_d[:])
            VC = 512
            nvc = (VSP + VC - 1) // VC
            for v in range(nvc):
                v0 = v * VC
                vn = min(VC, VSP - v0)
                for bb in range(B):
                    f0 = bb * FIN
                    r0 = erp.tile([FIN, VC], f32, tag="r0",
                                  name=f"r0_{v}_{bb}")
                    nc.sync.dma_start(
                        out=r0[:, :vn], in_=x0t[f0:f0 + FIN, v0:v0 + vn])
                    rk = {}
                    for k in (1, 2, 3):
                        rt_ = erp.tile([FIN, VC], bf16, tag=f"rk{k}",
                                       name=f"rk_{v}_{bb}_{k}")
                        nc.sync.dma_start(
                            out=rt_[:, :vn],
                            in_=xt[k - 1][f0:f0 + FIN, v0:v0 + vn])
                        rk[k] = rt_
                    ops = epp.tile([FOUT, VC], f32, tag=f"eps{bb % 4}",
                                   name=f"eps_{v}_{bb}")
                    nc.tensor.matmul(ops[:, :vn], w0_t[:], r0[:, :vn],
                                     start=True, stop=False)
                    for k in (1, 2, 3):
                        nc.tensor.matmul(ops[:, :vn], wb_t[:, k - 1, :],
                                         rk[k][:, :vn],
                                         start=False, stop=(k == 3))
                    ot = eop.tile([FOUT, VC], f32, tag="ot",
                                  name=f"ot_{v}_{bb}")
                    nc.vector.tensor_scalar(
                        ot[:, :vn], ops[:, :vn], bias_t[:], None,
                        op0=mybir.AluOpType.add)
                    nc.sync.dma_start(out=outT[bb][:, v0:v0 + vn],
                                      in_=ot[:, :vn])

    nc.compile()
    return nc


def _host_prep(x, weight, bias, lap_vals, lap_rows, lap_cols, cfg):
    NC, VS, VSP, VG, D, V = (cfg["NC"], cfg["VS"], cfg["VSP"], cfg["VG"],
                             cfg["D"], cfg["V"])
    x = np.asarray(x, dtype=np.float32)
    x0 = np.ascontiguousarray(x.transpose(1, 0, 2).reshape(V, D))

    prog, per_core = preprocess(lap_rows, lap_cols, lap_vals, cfg)

    x0p = np.zeros((VG, D), dtype=np.float32)
    for c in range(NC):
        x0p[c * VSP:c * VSP + VS] = x0[c * VS:(c + 1) * VS]
    xg0 = x0p.astype(BF16)

    iota = np.tile(np.arange(cfg["RT"], dtype=np.float32).astype(BF16),
                   (128, 1))
    ident = np.eye(128, dtype=np.float32)
    weight = np.asarray(weight, dtype=np.float32)
    w0 = weight[0]
    wb = np.zeros((cfg["FIN"], 3, cfg["FOUT"]), dtype=BF16)
    for k in (1, 2, 3):
        wb[:, k - 1] = weight[k].astype(BF16)
    bias_in = np.asarray(bias, dtype=np.float32).reshape(cfg["FOUT"], 1)

    in_maps = []
    for c in range(NC):
        x0sh = x0p[c * VSP:(c + 1) * VSP]
        in_maps.append({
            "xg0": xg0,
            "x0s": np.ascontiguousarray(x0sh),
            "x0t": np.ascontiguousarray(x0sh.T),
            "gidx": per_core[c]["gidx"],
            "meta": per_core[c]["meta"],
            "iota": iota,
            "ident": ident,
            "w0": w0,
            "wb": wb,
            "biasin": bias_in,
        })
    return prog, in_maps


def _assemble(results, cfg):
    NC, VS, VSP, B, FOUT, V = (cfg["NC"], cfg["VS"], cfg["VSP"], cfg["B"],
                               cfg["FOUT"], cfg["V"])
    out = np.empty((B, V, FOUT), dtype=np.float32)
    for c in range(NC):
        oT = np.asarray(results[c]["outT"]).reshape(B, FOUT, VSP)
        out[:, c * VS:(c + 1) * VS, :] = oT.transpose(0, 2, 1)[:, :VS, :]
    return out


def run(x, weight, bias, lap_vals, lap_rows, lap_cols, trace=False, cfg=None):
    from concourse import bass_utils

    cfg = cfg or make_cfg()
    prog, in_maps = _host_prep(x, weight, bias, lap_vals, lap_rows, lap_cols,
                               cfg)
    nc = build_nc(cfg, prog)
    res = bass_utils.run_bass_kernel_spmd(nc, in_maps, list(range(cfg["NC"])),
                                          trace=trace)
    return _assemble(res.results, cfg), res


def kernel(x, weight, bias, lap_vals, lap_rows, lap_cols):
    out, _ = run(x, weight, bias, lap_vals, lap_rows, lap_cols)
    return out


# revision 3
# speedup vs baseline: 1.0530x; 1.0530x over previous
"""ChebConv (K=4) GNN layer on 8 Trainium2 NeuronCores — v4.

Design (replaces the ap_gather/PE-transpose pipeline, which was
GPSIMD-bound at ~22ns/edge):
  - Nodes V row-sharded across 8 cores (VS rows, padded to VSP).
  - The current poly y lives replicated in DRAM as row-major bf16
    [VG, D] (D = B*FIN = 256 -> 512B rows).
  - Per row-tile cell, gpsimd.dma_gather pulls the edges' source rows
    straight from DRAM into SBUF in edge-partitioned layout
    [128, ngroups, D] (SWDGE descriptors; no PE transpose needed).
  - Segment-sum via one-hot matmul: S[e, r] = val[e] * (rloc[e] == r)
    generated on-chip from iota + meta, contracted against the gathered
    rows, accumulating a [128, D] psum per row-tile across its cells.
  - Chebyshev recurrence per row-tile; new shard written row-major bf16
    and AllGathered to rebuild the replica for the next round.
  - Final einsum contracts T_k^T with the replicated weight on the PE.

SPMD: per-cell edge counts are padded to the max across cores, so the
instruction stream is identical on all cores; only idx/meta data differ.
"""

import sys

import numpy as np

sys.path.insert(0, "/opt/trn_rl_repo")

import ml_dtypes  # noqa: E402

BF16 = ml_dtypes.bfloat16
FP8 = ml_dtypes.float8_e4m3fn


def make_cfg(V=100000, E=1600000, B=4, FIN=64, FOUT=64, NC=8, RT=128, NCH=4,
             NQ=1, PSB=4, gdt="fp8"):
    VS = V // NC
    assert VS * NC == V
    VSP = ((VS + RT - 1) // RT) * RT
    NT = VSP // RT
    VG = VSP * NC
    assert VG % NCH == 0
    CHUNK = VG // NCH
    assert CHUNK <= 32767  # idx is int16
    D = B * FIN
    return dict(V=V, E=E, B=B, FIN=FIN, FOUT=FOUT, NC=NC, RT=RT, NCH=NCH,
                CHUNK=CHUNK, VS=VS, VSP=VSP, NT=NT, VG=VG, D=D, NQ=NQ,
                PSB=PSB, gdt=gdt)


def _wrap16(idx, npart=128):
    n = idx.shape[0]
    w = idx.reshape(n // 16, 16).T  # [16, n/16]
    return np.tile(w, (npart // 16, 1))


def preprocess(rows, cols, vals, cfg):
    """Static SPMD schedule + per-core idx/meta data.

    prog: per row-tile, list of (ch, n16, ioff, goff) cells.
    per_core[c]: gidx [128, NIDX/16] i16, meta [128, NG, 2] f32.
    """
    NC, VS, VSP, RT, NT = cfg["NC"], cfg["VS"], cfg["VSP"], cfg["RT"], cfg["NT"]
    CHUNK, NCH = cfg["CHUNK"], cfg["NCH"]

    rows = np.asarray(rows, dtype=np.int64)
    cols = np.asarray(cols, dtype=np.int64)
    vals = np.asarray(vals, dtype=np.float32)

    owner = rows // VS
    lr = rows - owner * VS
    rt = lr // RT
    rloc = lr - rt * RT
    gc = (cols // VS) * VSP + (cols % VS)  # padded-global source index
    ch = gc // CHUNK
    ci = (gc - ch * CHUNK).astype(np.int64)

    cell_of = rt * NCH + ch
    ncells = NT * NCH
    counts = np.zeros((NC, ncells), dtype=np.int64)
    for c in range(NC):
        counts[c] = np.bincount(cell_of[owner == c], minlength=ncells)
    mx = counts.max(axis=0)
    n16 = ((mx + 15) // 16) * 16
    n16v = n16.reshape(NT, NCH)
    for t in range(NT):
        if n16v[t].sum() == 0:
            n16v[t, 0] = 16  # keep the psum chain non-empty

    per_core_cells = []
    for c in range(NC):
        m = owner == c
        order = np.argsort(cell_of[m], kind="stable")
        e_ci = ci[m][order]
        e_rloc = rloc[m][order]
        e_val = vals[m][order]
        e_cell = cell_of[m][order]
        starts = np.searchsorted(e_cell, np.arange(ncells))
        ends = np.searchsorted(e_cell, np.arange(ncells) + 1)
        per_core_cells.append((e_ci, e_rloc, e_val, starts, ends))

    NIDX = int(n16v.sum())
    NG = int(((n16v + RT - 1) // RT).sum())

    gidx = [np.zeros(NIDX, dtype=np.int16) for _ in range(NC)]
    oh = [np.zeros((128, NG, 128), dtype=np.float32) for _ in range(NC)]

    prog = []
    ioff = 0
    goff = 0
    for t in range(NT):
        tcells = []
        for chv in range(NCH):
            n = int(n16v[t, chv])
            if n == 0:
                continue
            ngrp = (n + RT - 1) // RT
            for c in range(NC):
                e_ci, e_rloc, e_val, starts, ends = per_core_cells[c]
                s_, e_ = starts[t * NCH + chv], ends[t * NCH + chv]
                k = e_ - s_
                gidx[c][ioff:ioff + k] = e_ci[s_:e_].astype(np.int16)
                ohv = np.zeros((ngrp * RT, 128), dtype=np.float32)
                ohv[np.arange(k), e_rloc[s_:e_]] = e_val[s_:e_]
                oh[c][:, goff:goff + ngrp, :] = (
                    ohv.reshape(ngrp, RT, 128).transpose(1, 0, 2))
            tcells.append({"ch": chv, "n16": n, "ngrp": ngrp,
                           "ioff": ioff, "goff": goff})
            ioff += n
            goff += ngrp
        prog.append(tcells)
    assert ioff == NIDX and goff == NG

    per_core = [{"gidx": _wrap16(gidx[c]), "oh": oh[c]} for c in range(NC)]

    return {"NIDX": NIDX, "NG": NG, "tiles": prog}, per_core


def build_nc(cfg, prog, ag_mode="collective"):
    import concourse.bacc as bacc
    import concourse.mybir as mybir
    import concourse.tile as tile

    NC, VSP, VG, D, RT, NT = (cfg["NC"], cfg["VSP"], cfg["VG"], cfg["D"],
                              cfg["RT"], cfg["NT"])
    CHUNK, B, FIN, FOUT = cfg["CHUNK"], cfg["B"], cfg["FIN"], cfg["FOUT"]
    NCH, NQ, PSB = cfg["NCH"], cfg["NQ"], cfg["PSB"]
    NIDX, NG = prog["NIDX"], prog["NG"]
    f32, bf16, i16 = mybir.dt.float32, mybir.dt.bfloat16, mybir.dt.int16
    gdt = bf16 if cfg["gdt"] == "bf16" else mybir.dt.float8e4
    AG_GROUPS = [list(range(NC))]
    GMAX = max(c["ngrp"] for tc in prog["tiles"] for c in tc)

    nc = bacc.Bacc("TRN2", target_bir_lowering=False, debug=False,
                   num_devices=NC)

    # inputs
    xg0 = nc.dram_tensor("xg0", [VG, D], gdt, kind="ExternalInput")
    x0s = nc.dram_tensor("x0s", [VSP, D], f32, kind="ExternalInput")
    x0t = nc.dram_tensor("x0t", [D, VSP], f32, kind="ExternalInput")
    gidx_d = nc.dram_tensor("gidx", [128, NIDX // 16], i16, kind="ExternalInput")
    oh_d = nc.dram_tensor("oh", [128, NG, 128], gdt, kind="ExternalInput")
    ident_d = nc.dram_tensor("ident", [128, 128], f32, kind="ExternalInput")
    w0_d = nc.dram_tensor("w0", [FIN, FOUT], f32, kind="ExternalInput")
    wb_d = nc.dram_tensor("wb", [FIN, 3, FOUT], bf16, kind="ExternalInput")
    bias_d = nc.dram_tensor("biasin", [FOUT, 1], f32, kind="ExternalInput")

    # outputs
    outT = nc.dram_tensor("outT", [B, FOUT, VSP], f32, kind="ExternalOutput")

    # internal DRAM
    xs1 = nc.dram_tensor("xs1", [VSP, D], f32)
    xb = [nc.dram_tensor(f"xb{k}", [VSP, D], gdt) for k in (1, 2)]
    yr = [nc.dram_tensor(f"yr{k}", [VG, D], gdt, addr_space="Shared")
          for k in (1, 2)]
    xt = [nc.dram_tensor(f"xt{k}", [D, VSP], bf16) for k in (1, 2, 3)]

    with tile.TileContext(nc) as tc:
        with (
            tc.tile_pool(name="static", bufs=1) as sp,
            tc.tile_pool(name="ztp", bufs=8) as ztp,
            tc.tile_pool(name="ohp", bufs=8) as ohp,
            tc.tile_pool(name="fin", bufs=3) as fp,
            tc.tile_pool(name="psum", bufs=1, space="PSUM") as pp,
            tc.tile_pool(name="pst", bufs=2, space="PSUM") as ppt,
        ):
            ident_t = sp.tile([128, 128], f32)
            gidx_t = sp.tile([128, NIDX // 16], i16)
            nc.sync.dma_start(out=ident_t[:], in_=ident_d[:])
            nc.sync.dma_start(out=gidx_t[:], in_=gidx_d[:])

            def spmm_round(r):
                src = xg0 if r == 1 else yr[r - 2]
                for t in range(NT):
                    cells = prog["tiles"][t]
                    tot = sum(c["ngrp"] for c in cells)
                    ps = pp.tile([128, D], f32, tag=f"ps{t % PSB}",
                                 name=f"ps_{r}_{t}")
                    g_i = 0
                    for cell in cells:
                        chv, n, ngrp = cell["ch"], cell["n16"], cell["ngrp"]
                        ioff, goff = cell["ioff"], cell["goff"]
                        zt = ztp.tile([128, GMAX, D], gdt, tag="zt",
                                      name=f"zt_{r}_{t}_{chv}")
                        nc.gpsimd.dma_gather(
                            zt[:, :ngrp, :],
                            src[chv * CHUNK:(chv + 1) * CHUNK, :],
                            gidx_t[:, ioff // 16:(ioff + n) // 16],
                            num_idxs=n, num_idxs_reg=n, elem_size=D,
                            queue_num=(t * NCH + chv) % NQ)
                        oh_t = ohp.tile([128, GMAX, 128], gdt, tag="oh",
                                        name=f"oh_{r}_{t}_{chv}")
                        nc.sync.dma_start(
                            out=oh_t[:, :ngrp, :],
                            in_=oh_d[:, goff:goff + ngrp, :])
                        for g in range(ngrp):
                            # last group only has kk valid gathered rows;
                            # contract over those to avoid reading the rest
                            kk = min(128, n - g * 128)
                            g_i += 1
                            nc.tensor.matmul(
                                ps[:], oh_t[:kk, g, :], zt[:kk, g, :],
                                start=(g_i == 1), stop=(g_i == tot))
                    # finalize row tile
                    xnew = fp.tile([128, D], f32, tag="xnew",
                                   name=f"xnew_{r}_{t}")
                    if r == 1:
                        nc.vector.tensor_copy(xnew[:], ps[:])
                        nc.sync.dma_start(
                            out=xs1[t * RT:(t + 1) * RT, :], in_=xnew[:])
                    else:
                        xprev_src = x0s if r == 2 else xs1
                        xp = fp.tile([128, D], f32, tag="xp",
                                     name=f"xp_{r}_{t}")
                        nc.sync.dma_start(
                            out=xp[:], in_=xprev_src[t * RT:(t + 1) * RT, :])
                        nc.vector.scalar_tensor_tensor(
                            xnew[:], ps[:], 2.0, xp[:],
                            op0=mybir.AluOpType.mult,
                            op1=mybir.AluOpType.subtract)
                    if r <= 2:
                        xnb = fp.tile([128, D], gdt, tag="xnb",
                                      name=f"xnb_{r}_{t}")
                        nc.scalar.copy(xnb[:], xnew[:])
                        nc.sync.dma_start(
                            out=xb[r - 1][t * RT:(t + 1) * RT, :], in_=xnb[:])
                    # transposed bf16 copy for the einsum
                    pa = ppt.tile([128, 128], f32, tag="pt",
                                  name=f"pa_{r}_{t}")
                    pb = ppt.tile([128, 128], f32, tag="pt",
                                  name=f"pb_{r}_{t}")
                    xe = xnew[:].rearrange("v (f two) -> v two f", two=2)
                    nc.tensor.transpose(pa[:], xe[:, 0, :], ident_t[:])
                    nc.tensor.transpose(pb[:], xe[:, 1, :], ident_t[:])
                    xtp = fp.tile([128, 2, 128], bf16, tag="xtp",
                                  name=f"xtp_{r}_{t}")
                    nc.any.tensor_copy(xtp[:, 0, :], pa[:])
                    nc.any.tensor_copy(xtp[:, 1, :], pb[:])
                    nc.sync.dma_start(
                        out=xt[r - 1].rearrange(
                            "(f two) v -> f two v",
                            two=2)[:, :, t * RT:(t + 1) * RT],
                        in_=xtp[:])
                if r <= 2:
                    if ag_mode == "collective":
                        nc.gpsimd.collective_compute(
                            "AllGather", mybir.AluOpType.bypass,
                            replica_groups=AG_GROUPS,
                            ins=[xb[r - 1][:]], outs=[yr[r - 1][:]])
                    else:  # single-core timing stand-in
                        for c in range(NC):
                            nc.sync.dma_start(
                                out=yr[r - 1][c * VSP:(c + 1) * VSP, :],
                                in_=xb[r - 1][:])

            for r in (1, 2, 3):
                spmm_round(r)

        # einsum: outT[b][o, v] = sum_k W_k^T @ T_k^T[b-rows, v] + bias
        with (
            tc.tile_pool(name="ew", bufs=1) as ewp,
            tc.tile_pool(name="erhs", bufs=3) as erp,
            tc.tile_pool(name="eout", bufs=3) as eop,
            tc.tile_pool(name="epsum", bufs=1, space="PSUM") as epp,
        ):
            w0_t = ewp.tile([FIN, FOUT], f32)
            wb_t = ewp.tile([FIN, 3, FOUT], bf16)
            bias_t = ewp.tile([FOUT, 1], f32)
            nc.sync.dma_start(out=w0_t[:], in_=w0_d[:])
            nc.sync.dma_start(out=wb_t[:], in_=wb_d[:])
            nc.sync.dma_start(out=bias_t[:], in_=bias_d[:])
            VC = 512
            nvc = (VSP + VC - 1) // VC
            for v in range(nvc):
                v0 = v * VC
                vn = min(VC, VSP - v0)
                for bb in range(B):
                    f0 = bb * FIN
                    r0 = erp.tile([FIN, VC], f32, tag="r0",
                                  name=f"r0_{v}_{bb}")
                    nc.sync.dma_start(
                        out=r0[:, :vn], in_=x0t[f0:f0 + FIN, v0:v0 + vn])
                    rk = {}
                    for k in (1, 2, 3):
                        rt_ = erp.tile([FIN, VC], bf16, tag=f"rk{k}",
                                       name=f"rk_{v}_{bb}_{k}")
                        nc.sync.dma_start(
                            out=rt_[:, :vn],
                            in_=xt[k - 1][f0:f0 + FIN, v0:v0 + vn])
                        rk[k] = rt_
                    ops = epp.tile([FOUT, VC], f32, tag=f"eps{bb % 4}",
                                   name=f"eps_{v}_{bb}")
                    nc.tensor.matmul(ops[:, :vn], w0_t[:], r0[:, :vn],
                                     start=True, stop=False)
                    for k in (1, 2, 3):
                        nc.tensor.matmul(ops[:, :vn], wb_t[:, k - 1, :],
                                         rk[k][:, :vn],
                                         start=False, stop=(k == 3))
                    ot = eop.tile([FOUT, VC], f32, tag="ot",
                                  name=f"ot_{v}_{bb}")
                    nc.vector.tensor_scalar(
                        ot[:, :vn], ops[:, :vn], bias_t[:], None,
                        op0=mybir.AluOpType.add)
                    nc.sync.dma_start(out=outT[bb][:, v0:v0 + vn],
                                      in_=ot[:, :vn])

    nc.compile()
    return nc


def _host_prep(x, weight, bias, lap_vals, lap_rows, lap_cols, cfg):
    NC, VS, VSP, VG, D, V = (cfg["NC"], cfg["VS"], cfg["VSP"], cfg["VG"],
                             cfg["D"], cfg["V"])
    GDT = BF16 if cfg["gdt"] == "bf16" else FP8
    x = np.asarray(x, dtype=np.float32)
    x0 = np.ascontiguousarray(x.transpose(1, 0, 2).reshape(V, D))

    prog, per_core = preprocess(lap_rows, lap_cols, lap_vals, cfg)

    x0p = np.zeros((VG, D), dtype=np.float32)
    for c in range(NC):
        x0p[c * VSP:c * VSP + VS] = x0[c * VS:(c + 1) * VS]
    xg0 = x0p.astype(GDT)

    ident = np.eye(128, dtype=np.float32)
    weight = np.asarray(weight, dtype=np.float32)
    w0 = weight[0]
    wb = np.zeros((cfg["FIN"], 3, cfg["FOUT"]), dtype=BF16)
    for k in (1, 2, 3):
        wb[:, k - 1] = weight[k].astype(BF16)
    bias_in = np.asarray(bias, dtype=np.float32).reshape(cfg["FOUT"], 1)

    in_maps = []
    for c in range(NC):
        x0sh = x0p[c * VSP:(c + 1) * VSP]
        in_maps.append({
            "xg0": xg0,
            "x0s": np.ascontiguousarray(x0sh),
            "x0t": np.ascontiguousarray(x0sh.T),
            "gidx": per_core[c]["gidx"],
            "oh": per_core[c]["oh"].astype(GDT),
            "ident": ident,
            "w0": w0,
            "wb": wb,
            "biasin": bias_in,
        })
    return prog, in_maps


def _assemble(results, cfg):
    NC, VS, VSP, B, FOUT, V = (cfg["NC"], cfg["VS"], cfg["VSP"], cfg["B"],
                               cfg["FOUT"], cfg["V"])
    out = np.empty((B, V, FOUT), dtype=np.float32)
    for c in range(NC):
        oT = np.asarray(results[c]["outT"]).reshape(B, FOUT, VSP)
        out[:, c * VS:(c + 1) * VS, :] = oT.transpose(0, 2, 1)[:, :VS, :]
    return out


def run(x, weight, bias, lap_vals, lap_rows, lap_cols, trace=False, cfg=None):
    from concourse import bass_utils

    cfg = cfg or make_cfg()
    prog, in_maps = _host_prep(x, weight, bias, lap_vals, lap_rows, lap_cols,
                               cfg)
    nc = build_nc(cfg, prog)
    res = bass_utils.run_bass_kernel_spmd(nc, in_maps, list(range(cfg["NC"])),
                                          trace=trace)
    return _assemble(res.results, cfg), res


def kernel(x, weight, bias, lap_vals, lap_rows, lap_cols):
    out, _ = run(x, weight, bias, lap_vals, lap_rows, lap_cols)
    return out
